# revision 12
# baseline (speedup 1.0000x reference)
"""Trainium2 Bass kernel for nn_BSplineActivationLayer.

Math:  y[b,o] = softplus( (1/OUT) * sum_i G[o,i] * f(x[b,i]; b1..b5[o,i]) )
where G = softplus(raw_gamma), b_s = piecewise-cubic spline of
w_norm = (clip(w,5.5,35.5)-20)/9, and
  f(x; b) = b1*log1p(b2*log1p((exp(b3*x)-1)**b4)) + b5*x.

Device algorithm (per core, OUT sharded 8 ways), tuned to the 2e-2 rel-err
budget (measured end-to-end error ~2e-3):
  * spline b_s is approximated piecewise-CONSTANT per piece (value of the
    cubic at the piece midpoint t=0.125); the per-element piece gather is a
    12-step masked multiply-accumulate per spline with the table values
    baked into the instruction stream as immediates (compile happens after
    inputs are seen; cache keyed on the table bytes).  clip() bounds prove
    pieces 0 and 14 unreachable, and the breaks are uniform so the masks
    compare raw w against MU+SIG*brk_j directly -- no normalization ops.
  * f is analytic in u = log(x); interpolate at NN=4 Chebyshev nodes in u:
      y[b,o] = softplus( (1/OUT) * [ sum_m  L_m(v[b,i]) @ N_m[o,i]
                                     + x @ (G*b5)[o,i] ] )
    with N_m = G*b1*cm_m*log1p(b2*log1p((exp(b3*x_m)-1)**b4)) node values
    and L_m the (unscaled) Lagrange basis products of v = norm(log x).
  * work is balanced across DVE (3 gather planes, chain muls, Lagrange
    finals, EN), Pool/GPSIMD (step masks, 2 gather planes, DD, products,
    gammas -- walrus accepts TensorTensor and immediate TensorScalar on
    Pool), and ACT (one manually placed set-6 table load serves every
    exp/ln/copy).  Matmul operands round to bf16 except the x-term, which
    stays f32 (PE has slack).  Junk matmuls keep PE ramped before the tail.
All value-dependent math on the big tensors runs on device; the host only
shards / transposes inputs, prepares the tiny (5x15) spline table constants,
and concatenates outputs.
"""

import numpy as np

B, IN, OUT = 256, 512, 512
NCORES = 8
OSH = OUT // NCORES            # 64 out-rows per core
NN = 4                         # interpolation nodes
NPIECE = 15
MU, SIG = 20.0, 9.0
U_LO, U_HI = float(np.log(0.01)), float(np.log(1.011))
TM = 0.125                     # piece-midpoint for the constant approx
JLO, JHI = 2, 13               # reachable step boundaries
NWARM = 0                      # junk matmuls to keep PE ramped (tuned)

_CACHE = {}


def _nodes():
    k = np.arange(NN)
    vn = np.cos((2 * k + 1) * np.pi / (2 * NN))          # in (-1, 1)
    xn = np.exp(0.5 * (U_HI + U_LO) + 0.5 * (U_HI - U_LO) * vn)
    cm = np.array([1.0 / np.prod(vn[m] - np.delete(vn, m)) for m in range(NN)])
    return vn, xn, cm


def _tables(breaks, coefs):
    """Host prep of the small spline tables -> immediates.

    Returns thr[j] (mask thresholds in raw-w domain, j=JLO..JHI),
    base[s], delta[s][j] for the piecewise-constant masked accumulate."""
    brk = breaks[0].astype(np.float64)
    cf = coefs.astype(np.float64)
    a3, a2, a1, a0 = cf[..., 0], cf[..., 1], cf[..., 2], cf[..., 3]
    vmid = ((a3 * TM + a2) * TM + a1) * TM + a0          # [NS, K]
    thr = MU + SIG * brk                                  # [16]
    base = vmid[:, 1]
    delta = vmid[:, 1:] - vmid[:, :-1]                    # delta[s, j-1] = v_j - v_{j-1}
    return thr, base, vmid, delta


def _emit(ctx, tc, yT, xT, wT, rgT, thr, base, delta):
    import concourse.bass as bass
    from concourse import mybir

    nc = tc.nc
    f32 = mybir.dt.float32
    bf16 = mybir.dt.bfloat16
    Alu = mybir.AluOpType
    Act = mybir.ActivationFunctionType
    vn, xn, cm = _nodes()

    P = 128
    IC = IN // P                      # 4 i-chunks
    FO = IC * OSH                     # 256
    FB = IC * B                       # 1024

    pool = ctx.enter_context(tc.tile_pool(name="main", bufs=1))
    pps = ctx.enter_context(tc.tile_pool(name="ps", bufs=1, space="PSUM"))

    def bcast_mid(ap2d, n):
        a = ap2d
        return bass.AP(tensor=a.tensor, offset=a.offset,
                       ap=[a.ap[0], [0, n], a.ap[1]])

    V = nc.vector
    Pl = nc.gpsimd
    S_ = nc.scalar

    ascale = 2.0 / (U_HI - U_LO)
    boff = (U_HI + U_LO) / (U_HI - U_LO)

    # ---- constants ---------------------------------------------------
    CP1 = pool.tile([P, 1], f32)
    V.memset(CP1, 1.0)
    CN1 = pool.tile([P, 1], f32)
    V.memset(CN1, -1.0)

    # ---- manual act-table load: set 6 covers exp/ln/copy -------------
    atl = mybir.InstLoadActFuncSet(
        name=nc.get_next_instruction_name(), act_func_set_id=6, ins=[], outs=[])
    S_.add_instruction(atl)

    # ---- DMAs --------------------------------------------------------
    W = pool.tile([P, FO], f32)
    nc.sync.dma_start(out=W.rearrange("p (c o) -> p c o", c=IC), in_=bass.AP(
        tensor=wT.tensor, offset=wT.offset,
        ap=[[OSH, P], [P * OSH, IC], [1, OSH]]))
    X = pool.tile([P, IC, B], f32)
    nc.sync.dma_start(out=X, in_=bass.AP(
        tensor=xT.tensor, offset=xT.offset,
        ap=[[B, P], [P * B, IC], [1, B]]))
    RG = pool.tile([P, FO], f32)
    nc.sync.dma_start(out=RG.rearrange("p (c o) -> p c o", c=IC), in_=bass.AP(
        tensor=rgT.tensor, offset=rgT.offset,
        ap=[[OSH, P], [P * OSH, IC], [1, OSH]]))

    # ---- step masks (Pool) ------------------------------------------
    NSTEP = JHI - JLO + 1             # 12
    ST = pool.tile([P, NSTEP, FO], f32)
    for j in range(JLO, JHI + 1):
        Pl.tensor_scalar(ST[:, j - JLO, :], W, float(thr[j]), 1.0,
                         Alu.is_gt, Alu.mult)

    # ---- gamma + log(x) (ACT) ---------------------------------------
    G = pool.tile([P, FO], f32)
    S_.activation(G, RG, Act.Exp)
    S_.activation(G, G, Act.Ln, bias=CP1)         # softplus(rg)
    U = pool.tile([P, IC, B], f32)
    XF = X.rearrange("p c b -> p (c b)")
    UF = U.rearrange("p c b -> p (c b)")
    S_.activation(UF, XF, Act.Ln)

    # ---- DD_m = v - vn_m on ACT (bf16), bf16 x copy for the x-term ---
    DD = pool.tile([P, NN, FB], bf16)
    for m in range(NN):
        S_.activation(DD[:, m, :], UF, Act.Copy, scale=ascale,
                      bias=-(boff + float(vn[m])))
    XB = pool.tile([P, IC, B], bf16)
    S_.activation(XB.rearrange("p c b -> p (c b)"), XF, Act.Copy)

    # ---- all 5 gather planes on DVE (TSP/STT has no perf modes; DVE is
    # still the cheapest engine for the masked accumulate).  Chain planes
    # first (b3,b4,b2), Lagrange finals, then b1 (gamma), b5 (x-term). --
    A = [pool.tile([P, FO], f32, name=f"A{s}") for s in range(5)]
    E = pool.tile([P, NN, FO], f32)
    EF = E.rearrange("p n f -> p (n f)")
    EB = pool.tile([P, NN, FO], bf16)
    P01 = pool.tile([P, FB], bf16)
    P23 = pool.tile([P, FB], bf16)
    LB = pool.tile([P, NN, FB], bf16)
    GB1 = pool.tile([P, FO], f32)
    GCM = pool.tile([P, NN, FO], bf16)
    GB5 = pool.tile([P, FO], bf16)

    def plane(s, after=None):
        """Masked accumulate for spline s.  `after` serializes this plane
        behind another plane's accumulator via a zero-mult init, so the
        scheduler cannot interleave it with earlier (more urgent) planes."""
        if after is None:
            V.tensor_scalar(A[s], ST[:, 0, :], float(delta[s, JLO - 1]),
                            float(base[s]), Alu.mult, Alu.add)
            j0 = JLO + 1
        else:
            V.tensor_scalar(A[s], A[after], 0.0, float(base[s]),
                            Alu.mult, Alu.add)
            j0 = JLO
        for j in range(j0, JHI + 1):
            V.scalar_tensor_tensor(A[s], ST[:, j - JLO, :],
                                   float(delta[s, j - 1]), A[s],
                                   Alu.mult, Alu.add)

    plane(2)
    # b3 ready: node exponentials + lam on ACT
    for m in range(NN):
        S_.activation(E[:, m, :], A[2], Act.Exp, scale=float(xn[m]))
    S_.activation(EF, EF, Act.Ln, bias=CN1)       # lam = ln(e^{b3 xm}-1)
    # Lagrange pairs on Pool as soon as DD lands
    Pl.tensor_tensor(P01, DD[:, 0, :], DD[:, 1, :], Alu.mult)
    Pl.tensor_tensor(P23, DD[:, 2, :], DD[:, 3, :], Alu.mult)

    plane(3)
    Pl.tensor_tensor(E, E, bcast_mid(A[3], NN), Alu.mult)   # T = lam*b4
    S_.activation(EF, EF, Act.Exp)                # (e^{b3 xm}-1)^{b4}
    S_.activation(EF, EF, Act.Ln, bias=CP1)       # L1 = log1p(...)

    plane(1, after=3)
    Pl.tensor_tensor(E, E, bcast_mid(A[1], NN), Alu.mult)   # b2*L1
    S_.activation(EB.rearrange("p n f -> p (n f)"), EF,
                  Act.Ln, bias=CP1)               # L2 = log1p(b2 L1)

    # Lagrange finals on DVE (bf16 2x) between chain planes and tail planes
    V.tensor_tensor(LB[:, 0, :], DD[:, 1, :], P23, Alu.mult)
    V.tensor_tensor(LB[:, 1, :], DD[:, 0, :], P23, Alu.mult)
    V.tensor_tensor(LB[:, 2, :], P01, DD[:, 3, :], Alu.mult)
    V.tensor_tensor(LB[:, 3, :], P01, DD[:, 2, :], Alu.mult)

    plane(0, after=1)
    Pl.tensor_tensor(GB1, G, A[0], Alu.mult)
    for m in range(NN):
        Pl.tensor_scalar(GCM[:, m, :], GB1, float(cm[m]), 1.0,
                         Alu.mult, Alu.mult)

    plane(4, after=0)
    Pl.tensor_tensor(GB5, G, A[4], Alu.mult)

    # ---- matmuls; EN split per node so PE starts as soon as possible -
    ps = pps.tile([OSH, B], f32)
    if NWARM:
        psj = pps.tile([OSH, B], f32)
        ZJ = pool.tile([P, OSH], bf16)
        ZM = pool.tile([P, B], bf16)
        V.memset(ZJ, 0.0)
        V.memset(ZM, 0.0)
        for k in range(NWARM):
            nc.tensor.matmul(psj, ZJ, ZM, start=(k == 0), stop=(k == NWARM - 1))
    nmm = IC * (NN + 1)
    k = 0
    EN = pool.tile([P, NN, FO], bf16)
    ENv = EN.rearrange("p n (c o) -> p n c o", c=IC)
    LBv = LB.rearrange("p n (c b) -> p n c b", c=IC)
    for m in range(NN):
        V.tensor_tensor(EN[:, m, :], EB[:, m, :], GCM[:, m, :], Alu.mult)
        for ic in range(IC):
            nc.tensor.matmul(ps, ENv[:, m, ic, :], LBv[:, m, ic, :],
                             start=(k == 0), stop=(k == nmm - 1))
            k += 1
    GB5v = GB5.rearrange("p (c o) -> p c o", c=IC)
    for ic in range(IC):
        nc.tensor.matmul(ps, GB5v[:, ic, :], XB[:, ic, :],
                         start=(k == 0), stop=(k == nmm - 1))
        k += 1

    # ---- softplus + store -------------------------------------------
    Y = pool.tile([OSH, B], f32)
    S_.activation(Y, ps, Act.Exp, scale=1.0 / OUT)
    S_.activation(Y, Y, Act.Ln, bias=CP1[0:OSH, :])
    nc.sync.dma_start(out=yT, in_=Y)


def _build(breaks, coefs):
    key = (breaks.tobytes(), coefs.tobytes())
    if key in _CACHE:
        return _CACHE[key]
    from contextlib import ExitStack
    import concourse.bacc as bacc
    import concourse.tile as tile
    from concourse import mybir

    thr, base, vmid, delta = _tables(breaks, coefs)

    f32 = mybir.dt.float32
    nc = bacc.Bacc("TRN2", target_bir_lowering=False, debug=False,
                   num_devices=NCORES)
    xT = nc.dram_tensor("xT", [IN, B], f32, kind="ExternalInput").ap()
    wT = nc.dram_tensor("wT", [IN, OSH], f32, kind="ExternalInput").ap()
    rgT = nc.dram_tensor("rgT", [IN, OSH], f32, kind="ExternalInput").ap()
    yT = nc.dram_tensor("yT", [OSH, B], f32, kind="ExternalOutput").ap()

    with tile.TileContext(nc) as tc, ExitStack() as ctx:
        _emit(ctx, tc, yT, xT, wT, rgT, thr, base, delta)
    nc.compile()
    _CACHE[key] = nc
    return nc


def _prep_inputs(x, raw_gamma, w, breaks, coefs):
    xT = np.ascontiguousarray(x.T, dtype=np.float32)
    maps = []
    for c in range(NCORES):
        o0, o1 = c * OSH, (c + 1) * OSH
        maps.append({
            "xT": xT,
            "wT": np.ascontiguousarray(w[o0:o1].T, dtype=np.float32),
            "rgT": np.ascontiguousarray(raw_gamma[o0:o1].T, dtype=np.float32),
        })
    return maps


def kernel(x, raw_gamma, w, breaks, coefs):
    from concourse.bass_utils import run_bass_kernel_spmd
    nc = _build(np.asarray(breaks), np.asarray(coefs))
    maps = _prep_inputs(x, raw_gamma, w, breaks, coefs)
    res = run_bass_kernel_spmd(nc, maps, list(range(NCORES)))
    y = np.concatenate([res.results[c]["yT"].T for c in range(NCORES)], axis=1)
    return np.ascontiguousarray(y, dtype=np.float32)


# revision 16
# speedup vs baseline: 1.1107x; 1.1107x over previous
"""Trainium2 Bass kernel for nn_BSplineActivationLayer.

Math:  y[b,o] = softplus( (1/OUT) * sum_i G[o,i] * f(x[b,i]; b1..b5[o,i]) )
where G = softplus(raw_gamma), b_s = piecewise-cubic spline of
w_norm = (clip(w,5.5,35.5)-20)/9, and
  f(x; b) = b1*log1p(b2*log1p((exp(b3*x)-1)**b4)) + b5*x.

Device algorithm (per core, OUT sharded 8 ways), tuned to the 2e-2 rel-err
budget (measured end-to-end error ~2e-3):
  * spline b_s is approximated piecewise-CONSTANT per piece (value of the
    cubic at the piece midpoint t=0.125); the per-element piece gather is a
    12-step masked multiply-accumulate per spline with the table values
    baked into the instruction stream as immediates (compile happens after
    inputs are seen; cache keyed on the table bytes).  clip() bounds prove
    pieces 0 and 14 unreachable, and the breaks are uniform so the masks
    compare raw w against MU+SIG*brk_j directly -- no normalization ops.
  * f is analytic in u = log(x); interpolate at NN=4 Chebyshev nodes in u:
      y[b,o] = softplus( (1/OUT) * [ sum_m  L_m(v[b,i]) @ N_m[o,i]
                                     + x @ (G*b5)[o,i] ] )
    with N_m = G*b1*cm_m*log1p(b2*log1p((exp(b3*x_m)-1)**b4)) node values
    and L_m the (unscaled) Lagrange basis products of v = norm(log x).
  * work is balanced across DVE (3 gather planes, chain muls, Lagrange
    finals, EN), Pool/GPSIMD (step masks, 2 gather planes, DD, products,
    gammas -- walrus accepts TensorTensor and immediate TensorScalar on
    Pool), and ACT (one manually placed set-6 table load serves every
    exp/ln/copy).  Matmul operands round to bf16 except the x-term, which
    stays f32 (PE has slack).  Junk matmuls keep PE ramped before the tail.
All value-dependent math on the big tensors runs on device; the host only
shards / transposes inputs, prepares the tiny (5x15) spline table constants,
and concatenates outputs.
"""

import numpy as np

B, IN, OUT = 256, 512, 512
NCORES = 8
OSH = OUT // NCORES            # 64 out-rows per core
NN = 4                         # interpolation nodes
NPIECE = 15
MU, SIG = 20.0, 9.0
U_LO, U_HI = float(np.log(0.01)), float(np.log(1.011))
TM = 0.125                     # piece-midpoint for the constant approx
JLO, JHI = 2, 13               # reachable step boundaries
NWARM = 0                      # junk matmuls to keep PE ramped (tuned)

_CACHE = {}


def _nodes():
    k = np.arange(NN)
    vn = np.cos((2 * k + 1) * np.pi / (2 * NN))          # in (-1, 1)
    xn = np.exp(0.5 * (U_HI + U_LO) + 0.5 * (U_HI - U_LO) * vn)
    cm = np.array([1.0 / np.prod(vn[m] - np.delete(vn, m)) for m in range(NN)])
    return vn, xn, cm


def _tables(breaks, coefs):
    """Host prep of the small spline tables -> immediates.

    Returns thr[j] (mask thresholds in raw-w domain, j=JLO..JHI),
    base[s], delta[s][j] for the piecewise-constant masked accumulate."""
    brk = breaks[0].astype(np.float64)
    cf = coefs.astype(np.float64)
    a3, a2, a1, a0 = cf[..., 0], cf[..., 1], cf[..., 2], cf[..., 3]
    vmid = ((a3 * TM + a2) * TM + a1) * TM + a0          # [NS, K]
    thr = MU + SIG * brk                                  # [16]
    base = vmid[:, 1]
    delta = vmid[:, 1:] - vmid[:, :-1]                    # delta[s, j-1] = v_j - v_{j-1}
    return thr, base, vmid, delta


def _emit(ctx, tc, yT, xT, wT, rgT, thr, base, delta):
    import concourse.bass as bass
    from concourse import mybir

    nc = tc.nc
    f32 = mybir.dt.float32
    bf16 = mybir.dt.bfloat16
    Alu = mybir.AluOpType
    Act = mybir.ActivationFunctionType
    vn, xn, cm = _nodes()

    P = 128
    IC = IN // P                      # 4 i-chunks
    FO = IC * OSH                     # 256
    FB = IC * B                       # 1024

    pool = ctx.enter_context(tc.tile_pool(name="main", bufs=1))
    pps = ctx.enter_context(tc.tile_pool(name="ps", bufs=1, space="PSUM"))

    def bcast_mid(ap2d, n):
        a = ap2d
        return bass.AP(tensor=a.tensor, offset=a.offset,
                       ap=[a.ap[0], [0, n], a.ap[1]])

    V = nc.vector
    Pl = nc.gpsimd
    S_ = nc.scalar

    ascale = 2.0 / (U_HI - U_LO)
    boff = (U_HI + U_LO) / (U_HI - U_LO)

    # ---- constants ---------------------------------------------------
    CP1 = pool.tile([P, 1], f32)
    V.memset(CP1, 1.0)
    CN1 = pool.tile([P, 1], f32)
    V.memset(CN1, -1.0)

    # ---- manual act-table load: set 6 covers exp/ln/copy -------------
    atl = mybir.InstLoadActFuncSet(
        name=nc.get_next_instruction_name(), act_func_set_id=6, ins=[], outs=[])
    S_.add_instruction(atl)

    # ---- DMAs --------------------------------------------------------
    W = pool.tile([P, FO], f32)
    nc.sync.dma_start(out=W.rearrange("p (c o) -> p c o", c=IC), in_=bass.AP(
        tensor=wT.tensor, offset=wT.offset,
        ap=[[OSH, P], [P * OSH, IC], [1, OSH]]))
    X = pool.tile([P, IC, B], f32)
    nc.sync.dma_start(out=X, in_=bass.AP(
        tensor=xT.tensor, offset=xT.offset,
        ap=[[B, P], [P * B, IC], [1, B]]))
    RG = pool.tile([P, FO], f32)
    nc.sync.dma_start(out=RG.rearrange("p (c o) -> p c o", c=IC), in_=bass.AP(
        tensor=rgT.tensor, offset=rgT.offset,
        ap=[[OSH, P], [P * OSH, IC], [1, OSH]]))

    # ---- step masks (Pool) ------------------------------------------
    NSTEP = JHI - JLO + 1             # 12
    ST = pool.tile([P, NSTEP, FO], f32)
    for j in range(JLO, JHI + 1):
        Pl.tensor_scalar(ST[:, j - JLO, :], W, float(thr[j]), 1.0,
                         Alu.is_gt, Alu.mult)

    # ---- gamma + log(x) (ACT) ---------------------------------------
    G = pool.tile([P, FO], f32)
    S_.activation(G, RG, Act.Exp)
    S_.activation(G, G, Act.Ln, bias=CP1)         # softplus(rg)
    U = pool.tile([P, IC, B], f32)
    XF = X.rearrange("p c b -> p (c b)")
    UF = U.rearrange("p c b -> p (c b)")
    S_.activation(UF, XF, Act.Ln)

    # ---- DD_m = v - vn_m on ACT (bf16), bf16 x copy for the x-term ---
    DD = pool.tile([P, NN, FB], bf16)
    for m in range(NN):
        S_.activation(DD[:, m, :], UF, Act.Copy, scale=ascale,
                      bias=-(boff + float(vn[m])))
    XB = pool.tile([P, IC, B], bf16)
    S_.activation(XB.rearrange("p c b -> p (c b)"), XF, Act.Copy)

    # ---- all 5 gather planes on DVE (TSP/STT has no perf modes; DVE is
    # still the cheapest engine for the masked accumulate).  Chain planes
    # first (b3,b4,b2), Lagrange finals, then b1 (gamma), b5 (x-term). --
    A = [pool.tile([P, FO], f32, name=f"A{s}") for s in range(5)]
    E = pool.tile([P, NN, FO], f32)
    EF = E.rearrange("p n f -> p (n f)")
    EB = pool.tile([P, NN, FO], bf16)
    P01 = pool.tile([P, FB], bf16)
    P23 = pool.tile([P, FB], bf16)
    LB = pool.tile([P, NN, FB], bf16)
    GB1 = pool.tile([P, FO], f32)
    GCM = pool.tile([P, NN, FO], bf16)
    GB5 = pool.tile([P, FO], bf16)

    def plane(s, after=None):
        """Masked accumulate for spline s.  `after` serializes this plane
        behind another plane's accumulator via a zero-mult init, so the
        scheduler cannot interleave it with earlier (more urgent) planes."""
        if after is None:
            V.tensor_scalar(A[s], ST[:, 0, :], float(delta[s, JLO - 1]),
                            float(base[s]), Alu.mult, Alu.add)
            j0 = JLO + 1
        else:
            V.tensor_scalar(A[s], A[after], 0.0, float(base[s]),
                            Alu.mult, Alu.add)
            j0 = JLO
        for j in range(j0, JHI + 1):
            V.scalar_tensor_tensor(A[s], ST[:, j - JLO, :],
                                   float(delta[s, j - 1]), A[s],
                                   Alu.mult, Alu.add)

    plane(2)
    # b3 ready: node exponentials + lam on ACT
    for m in range(NN):
        S_.activation(E[:, m, :], A[2], Act.Exp, scale=float(xn[m]))
    S_.activation(EF, EF, Act.Ln, bias=CN1)       # lam = ln(e^{b3 xm}-1)
    # Lagrange pairs on Pool as soon as DD lands
    Pl.tensor_tensor(P01, DD[:, 0, :], DD[:, 1, :], Alu.mult)
    Pl.tensor_tensor(P23, DD[:, 2, :], DD[:, 3, :], Alu.mult)

    plane(3)
    Pl.tensor_tensor(E, E, bcast_mid(A[3], NN), Alu.mult)   # T = lam*b4
    S_.activation(EF, EF, Act.Exp)                # (e^{b3 xm}-1)^{b4}
    S_.activation(EF, EF, Act.Ln, bias=CP1)       # L1 = log1p(...)

    plane(1)
    Pl.tensor_tensor(E, E, bcast_mid(A[1], NN), Alu.mult)   # b2*L1
    S_.activation(EB.rearrange("p n f -> p (n f)"), EF,
                  Act.Ln, bias=CP1)               # L2 = log1p(b2 L1)

    # Lagrange finals on DVE (bf16 2x) between chain planes and tail planes
    V.tensor_tensor(LB[:, 0, :], DD[:, 1, :], P23, Alu.mult)
    V.tensor_tensor(LB[:, 1, :], DD[:, 0, :], P23, Alu.mult)
    V.tensor_tensor(LB[:, 2, :], P01, DD[:, 3, :], Alu.mult)
    V.tensor_tensor(LB[:, 3, :], P01, DD[:, 2, :], Alu.mult)

    plane(0)
    Pl.tensor_tensor(GB1, G, A[0], Alu.mult)
    for m in range(NN):
        Pl.tensor_scalar(GCM[:, m, :], GB1, float(cm[m]), 1.0,
                         Alu.mult, Alu.mult)

    plane(4)
    Pl.tensor_tensor(GB5, G, A[4], Alu.mult)

    # ---- matmuls; PSUM split by B-halves so softplus+store of half 0
    # overlaps half 1's matmuls; junk matmuls keep the PE p-state ramped
    # until the real ones arrive.
    HB = B // 2
    ps0 = pps.tile([OSH, HB], f32)
    ps1 = pps.tile([OSH, HB], f32)
    if NWARM:
        psj = pps.tile([OSH, 512], f32)
        ZJ = pool.tile([P, OSH], bf16)
        ZM = pool.tile([P, 512], bf16)
        V.memset(ZJ, 0.0)
        V.memset(ZM, 0.0)
        for k in range(NWARM):
            nc.tensor.matmul(psj, ZJ, ZM, start=(k == 0), stop=(k == NWARM - 1))
    nmm = IC * (NN + 1)
    EN = pool.tile([P, NN, FO], bf16)
    ENv = EN.rearrange("p n (c o) -> p n c o", c=IC)
    LBv = LB.rearrange("p n (c b) -> p n c b", c=IC)
    GB5v = GB5.rearrange("p (c o) -> p c o", c=IC)
    for m in range(NN):
        V.tensor_tensor(EN[:, m, :], EB[:, m, :], GCM[:, m, :], Alu.mult)
    for h, psh in ((0, ps0), (1, ps1)):
        lo, hi = h * HB, (h + 1) * HB
        k = 0
        for m in range(NN):
            for ic in range(IC):
                nc.tensor.matmul(psh, ENv[:, m, ic, :], LBv[:, m, ic, lo:hi],
                                 start=(k == 0), stop=(k == nmm - 1))
                k += 1
        for ic in range(IC):
            nc.tensor.matmul(psh, GB5v[:, ic, :], XB[:, ic, lo:hi],
                             start=(k == 0), stop=(k == nmm - 1))
            k += 1

    # ---- softplus + store, per half ---------------------------------
    Y = pool.tile([OSH, B], f32)
    for h, psh in ((0, ps0), (1, ps1)):
        lo, hi = h * HB, (h + 1) * HB
        S_.activation(Y[:, lo:hi], psh, Act.Exp, scale=1.0 / OUT)
        S_.activation(Y[:, lo:hi], Y[:, lo:hi], Act.Ln, bias=CP1[0:OSH, :])
        nc.sync.dma_start(out=bass.AP(tensor=yT.tensor, offset=yT.offset + lo,
                                      ap=[yT.ap[0], [1, HB]]),
                          in_=Y[:, lo:hi])


def _build(breaks, coefs):
    key = (breaks.tobytes(), coefs.tobytes())
    if key in _CACHE:
        return _CACHE[key]
    from contextlib import ExitStack
    import concourse.bacc as bacc
    import concourse.tile as tile
    from concourse import mybir

    thr, base, vmid, delta = _tables(breaks, coefs)

    f32 = mybir.dt.float32
    nc = bacc.Bacc("TRN2", target_bir_lowering=False, debug=False,
                   num_devices=NCORES)
    xT = nc.dram_tensor("xT", [IN, B], f32, kind="ExternalInput").ap()
    wT = nc.dram_tensor("wT", [IN, OSH], f32, kind="ExternalInput").ap()
    rgT = nc.dram_tensor("rgT", [IN, OSH], f32, kind="ExternalInput").ap()
    yT = nc.dram_tensor("yT", [OSH, B], f32, kind="ExternalOutput").ap()

    with tile.TileContext(nc) as tc, ExitStack() as ctx:
        _emit(ctx, tc, yT, xT, wT, rgT, thr, base, delta)
    nc.compile()
    _CACHE[key] = nc
    return nc


def _prep_inputs(x, raw_gamma, w, breaks, coefs):
    xT = np.ascontiguousarray(x.T, dtype=np.float32)
    maps = []
    for c in range(NCORES):
        o0, o1 = c * OSH, (c + 1) * OSH
        maps.append({
            "xT": xT,
            "wT": np.ascontiguousarray(w[o0:o1].T, dtype=np.float32),
            "rgT": np.ascontiguousarray(raw_gamma[o0:o1].T, dtype=np.float32),
        })
    return maps


def kernel(x, raw_gamma, w, breaks, coefs):
    from concourse.bass_utils import run_bass_kernel_spmd
    nc = _build(np.asarray(breaks), np.asarray(coefs))
    maps = _prep_inputs(x, raw_gamma, w, breaks, coefs)
    res = run_bass_kernel_spmd(nc, maps, list(range(NCORES)))
    y = np.concatenate([res.results[c]["yT"].T for c in range(NCORES)], axis=1)
    return np.ascontiguousarray(y, dtype=np.float32)
